# revision 21
# baseline (speedup 1.0000x reference)
"""KoLeo loss kernel for Trainium2 (8 NeuronCores, SPMD).

Strategy (v1.6: fp8 DoubleRow + flipped orientation + STT accumulator):
  - Shard rows of student_output [8192, 768] across 8 cores (1024 rows each).
  - Flipped Gram orientation: psum[p, f] = -2 x_j . x_i with j (all 8192
    rows, 64 j-tiles of 128) on PARTITIONS and i (the core's 1024 rows,
    2 i-tiles of 512) on the FREE axis, via fp8(e4m3) DoubleRow matmuls.
  - Row-min accumulation is then ONE cheap DVE op per psum tile:
      rowacc[p, i] = min(rowacc[p, i], psum[p, i] + sq_j[p])
    (scalar_tensor_tensor: the +sq_j is a per-partition scalar, so no
    free-axis-varying add and no per-tile reduce is needed).
  - Self-distance is masked by adding +BIG to the 8 diagonal psum tiles.
  - Final fold: PE-transpose rowacc chunks, DVE min-reduce -> [128, 8],
    host adds sq_i and does -mean(log(sqrt(d2)+eps)).
  - Per-core inputs are column-ROTATED by the core's row offset so the same
    SPMD program works on every core.
"""

import os

import numpy as np

try:
    import concourse  # noqa: F401
except ImportError:  # pragma: no cover - harness env fallback
    import sys

    sys.path.insert(0, "/opt/trn_rl_repo")

import concourse.bacc as bacc
import concourse.tile as tile
from concourse import mybir
from concourse.bass_utils import run_bass_kernel_spmd
from concourse.masks import make_identity

N = 8192
D = 768
NCORES = 8
ROWS_PER_CORE = N // NCORES  # 1024
KCH = D // 128  # 6 k-chunks of 128
KP = KCH // 2  # 3 DoubleRow k-pairs of 256
JT_PER_Q = 16  # j-tiles (128 wide) per 2048-column q block
NJT = N // 128  # 64 j-tiles total
IT = ROWS_PER_CORE // 512  # 2 i-tiles of 512
QB = 4  # j blocks of 2048
BIG = 30000.0  # diag mask; keeps every value finite in fp16
ACC_INIT = 60000.0
EPS = 1e-8

TRACE = os.environ.get("KOLEO_TRACE", "0") == "1"
USE_DR = os.environ.get("KOLEO_DR", "1") == "1"  # DoubleRow fp8 perf mode
DEV_FOLD = os.environ.get("KOLEO_DEV_FOLD", "1") == "1"  # fold on device
PROBE_PE = os.environ.get("KOLEO_PROBE_PE", "0") == "1"  # timing probe: matmuls only
LAST = None  # BassKernelResults stash for test harness

_NC = None


def build_nc(reps: int = 1):
    f32 = mybir.dt.float32
    f16 = mybir.dt.float16
    f8 = mybir.dt.float8e4

    nc = bacc.Bacc("TRN2", target_bir_lowering=False, debug=False, num_devices=NCORES)

    xt_d = nc.declare_dram_parameter("xt", [KCH, 128, N], f8, isOutput=False)
    xts_d = nc.declare_dram_parameter(
        "xts", [KCH, 128, ROWS_PER_CORE], f8, isOutput=False
    )
    sqj_d = nc.declare_dram_parameter("sqj", [128, NJT], f32, isOutput=False)
    bigdiag_d = nc.declare_dram_parameter("bigdiag", [4, 128, 512], f32, isOutput=False)
    if DEV_FOLD:
        minred_d = nc.declare_dram_parameter("minred", [128, 8], f32, isOutput=True)
    else:
        rowacc_d = nc.declare_dram_parameter(
            "rowacc", [128, IT, 512], f32, isOutput=True
        )

    with tile.TileContext(nc) as tc:
        with (
            tc.tile_pool(name="const", bufs=1) as cpool,
            tc.tile_pool(name="psum", bufs=3, space="PSUM") as psum_pool,
            tc.tile_pool(name="tpsum", bufs=2, space="PSUM") as tpsum_pool,
        ):
            # --- persistent SBUF tiles ---
            xts_t = cpool.tile([128, KCH, ROWS_PER_CORE], f8, tag="xts")
            for k in range(KCH):
                nc.sync.dma_start(xts_t[:, k], xts_d[k])

            sqj_t = cpool.tile([128, NJT], f32, tag="sqj")
            nc.sync.dma_start(sqj_t[:], sqj_d[:])

            bigdiag_t = []
            for b in range(4):
                t = cpool.tile([128, 512], f32, tag=f"bd{b}")
                nc.sync.dma_start(t[:], bigdiag_d[b])
                bigdiag_t.append(t)

            xt_t = []
            for q in range(QB):
                xq = cpool.tile([128, KCH, 2048], f8, tag=f"xt{q}")
                for k in range(KCH):
                    nc.sync.dma_start(xq[:, k], xt_d[k, :, q * 2048 : (q + 1) * 2048])
                xt_t.append(xq)

            rowacc = cpool.tile([128, IT, 512], f16, tag="rowacc")
            if DEV_FOLD:
                ident = cpool.tile([128, 128], f16, tag="ident")
                make_identity(nc, ident[:])
                minred_t = cpool.tile([128, 8], f32, tag="minredt")

            # --- main compute ---
            def body(_i=None):
                nc.vector.memset(rowacc[:], ACC_INIT)
                for q in range(QB):
                    for jt in range(JT_PER_Q):
                        JT = q * JT_PER_Q + jt
                        # one two-bank psum tile per j-tile group: both i-tiles
                        # side by side, consumed by a single wide STT below.
                        ps = psum_pool.tile([128, IT, 512], f32, tag="ps")
                        if USE_DR:
                            for kp in range(KP):
                                lhs = xt_t[q][
                                    :, 2 * kp : 2 * kp + 2, jt * 128 : (jt + 1) * 128
                                ]
                                for it in range(IT):
                                    nc.tensor.matmul(
                                        ps[:, it],
                                        lhs,
                                        xts_t[
                                            :,
                                            2 * kp : 2 * kp + 2,
                                            it * 512 : (it + 1) * 512,
                                        ],
                                        start=(kp == 0),
                                        stop=(kp == KP - 1),
                                        perf_mode=mybir.MatmulPerfMode.DoubleRow,
                                    )
                        else:
                            for k in range(KCH):
                                lhs = xt_t[q][:, k, jt * 128 : (jt + 1) * 128]
                                for it in range(IT):
                                    nc.tensor.matmul(
                                        ps[:, it],
                                        lhs,
                                        xts_t[:, k, it * 512 : (it + 1) * 512],
                                        start=(k == 0),
                                        stop=(k == KCH - 1),
                                    )
                        if PROBE_PE:
                            if JT % 8 == 7:
                                # minimal psum consumer so tiles recycle
                                nc.vector.scalar_tensor_tensor(
                                    rowacc[:],
                                    ps[:],
                                    sqj_t[:, JT : JT + 1],
                                    rowacc[:],
                                    op0=mybir.AluOpType.add,
                                    op1=mybir.AluOpType.min,
                                )
                            continue
                        # self-distance: +BIG on the diagonal psum slice
                        if JT < 8:
                            nc.vector.tensor_tensor(
                                ps[:, JT // 4],
                                ps[:, JT // 4],
                                bigdiag_t[JT % 4][:],
                                op=mybir.AluOpType.add,
                            )
                        nc.vector.scalar_tensor_tensor(
                            rowacc[:],
                            ps[:],
                            sqj_t[:, JT : JT + 1],
                            rowacc[:],
                            op0=mybir.AluOpType.add,
                            op1=mybir.AluOpType.min,
                        )

                if DEV_FOLD:
                    for chunk in range(8):
                        it, cc = divmod(chunk, 4)
                        tp = tpsum_pool.tile([128, 128], f16, tag="tp", name="tp")
                        nc.tensor.transpose(
                            tp[:], rowacc[:, it, cc * 128 : (cc + 1) * 128], ident[:]
                        )
                        nc.vector.tensor_reduce(
                            minred_t[:, chunk : chunk + 1],
                            tp[:],
                            axis=mybir.AxisListType.X,
                            op=mybir.AluOpType.min,
                        )

            if reps == 1:
                body()
            else:
                with tc.For_i(0, reps, 1) as _i:
                    body(_i)

            if DEV_FOLD:
                nc.sync.dma_start(minred_d[:], minred_t[:])
            else:
                acc_f32 = cpool.tile([128, IT, 512], f32, tag="accf32")
                nc.vector.tensor_copy(acc_f32[:], rowacc[:])
                nc.sync.dma_start(rowacc_d[:], acc_f32[:])

    nc.compile()
    return nc


def make_in_maps(x: np.ndarray):
    import ml_dtypes

    f8 = ml_dtypes.float8_e4m3
    sq = np.einsum("nd,nd->n", x.astype(np.float64), x.astype(np.float64)).astype(
        np.float32
    )  # [N]
    xt8 = np.ascontiguousarray(x.T).astype(f8)  # [D, N]

    bigdiag = np.zeros((4, 128, 512), np.float32)
    for b in range(4):
        bigdiag[b, np.arange(128), b * 128 + np.arange(128)] = BIG

    in_maps = []
    for c in range(NCORES):
        shift = c * ROWS_PER_CORE
        xt_rot = np.ascontiguousarray(np.roll(xt8, -shift, axis=1)).reshape(
            KCH, 128, N
        )
        sq_rot = np.roll(sq, -shift)
        # sqj[p, JT] = sq of global row for local column JT*128+p
        sqj = np.ascontiguousarray(sq_rot.reshape(NJT, 128).T)
        xts = (
            np.ascontiguousarray((-2.0 * x[shift : shift + ROWS_PER_CORE].T))
            .astype(f8)
            .reshape(KCH, 128, ROWS_PER_CORE)
        )
        in_maps.append({"xt": xt_rot, "xts": xts, "sqj": sqj, "bigdiag": bigdiag})
    return in_maps, sq


# back-compat aliases for the timing harness
_build_nc = build_nc
_make_in_maps = make_in_maps


def kernel(student_output: np.ndarray) -> np.ndarray:
    global _NC, LAST

    x = np.asarray(student_output, dtype=np.float32)
    assert x.shape == (N, D)
    in_maps, sq = make_in_maps(x)

    if _NC is None:
        _NC = build_nc()

    res = run_bass_kernel_spmd(_NC, in_maps, list(range(NCORES)), trace=TRACE)
    LAST = res
    results = res.results

    percore = []
    for c in range(NCORES):
        if DEV_FOLD:
            mr = np.asarray(results[c]["minred"], np.float32)  # [128, 8]
            percore.append(mr.T.reshape(-1))  # local i = chunk*128 + p
        else:
            ra = np.asarray(results[c]["rowacc"], np.float32)  # [128, IT, 512]
            # min over j-partitions: candidate[p, it, f] -> local i = it*512+f
            percore.append(ra.min(axis=0).reshape(-1))
    mins = np.concatenate(percore)  # [N] ordered by global row
    d2 = np.maximum(mins.astype(np.float64) + sq.astype(np.float64), 0.0)
    val = -np.mean(np.log(np.sqrt(d2) + EPS))
    return np.array(val, dtype=np.float32)


# revision 24
# speedup vs baseline: 1.1192x; 1.1192x over previous
"""KoLeo loss kernel for Trainium2 (8 NeuronCores, SPMD).

Strategy (v1.6: fp8 DoubleRow + flipped orientation + STT accumulator):
  - Shard rows of student_output [8192, 768] across 8 cores (1024 rows each).
  - Flipped Gram orientation: psum[p, f] = -2 x_j . x_i with j (all 8192
    rows, 64 j-tiles of 128) on PARTITIONS and i (the core's 1024 rows,
    2 i-tiles of 512) on the FREE axis, via fp8(e4m3) DoubleRow matmuls.
  - Row-min accumulation is then ONE cheap DVE op per psum tile:
      rowacc[p, i] = min(rowacc[p, i], psum[p, i] + sq_j[p])
    (scalar_tensor_tensor: the +sq_j is a per-partition scalar, so no
    free-axis-varying add and no per-tile reduce is needed).
  - Self-distance is masked by adding +BIG to the 8 diagonal psum tiles.
  - Final fold: PE-transpose rowacc chunks, DVE min-reduce -> [128, 8],
    host adds sq_i and does -mean(log(sqrt(d2)+eps)).
  - Per-core inputs are column-ROTATED by the core's row offset so the same
    SPMD program works on every core.
"""

import os

import numpy as np

try:
    import concourse  # noqa: F401
except ImportError:  # pragma: no cover - harness env fallback
    import sys

    sys.path.insert(0, "/opt/trn_rl_repo")

import concourse.bacc as bacc
import concourse.tile as tile
from concourse import mybir
from concourse.bass_utils import run_bass_kernel_spmd
from concourse.masks import make_identity

N = 8192
D = 768
NCORES = 8
ROWS_PER_CORE = N // NCORES  # 1024
KCH = D // 128  # 6 k-chunks of 128
KP = KCH // 2  # 3 DoubleRow k-pairs of 256
JT_PER_Q = 16  # j-tiles (128 wide) per 2048-column q block
NJT = N // 128  # 64 j-tiles total
IT = ROWS_PER_CORE // 512  # 2 i-tiles of 512
QB = 4  # j blocks of 2048
BIG = 30000.0  # diag mask; keeps every value finite in fp16
ACC_INIT = 60000.0
EPS = 1e-8

TRACE = os.environ.get("KOLEO_TRACE", "0") == "1"
USE_DR = os.environ.get("KOLEO_DR", "1") == "1"  # DoubleRow fp8 perf mode
DEV_FOLD = os.environ.get("KOLEO_DEV_FOLD", "1") == "1"  # fold on device
PROBE_PE = os.environ.get("KOLEO_PROBE_PE", "0") == "1"  # timing probe: matmuls only
DMA_IN_LOOP = os.environ.get("KOLEO_DMA_IN_LOOP", "0") == "1"  # timing probe: re-DMA inputs per rep
LAST = None  # BassKernelResults stash for test harness

_NC = None


def build_nc(reps: int = 1):
    f32 = mybir.dt.float32
    f16 = mybir.dt.float16
    f8 = mybir.dt.float8e4

    nc = bacc.Bacc("TRN2", target_bir_lowering=False, debug=False, num_devices=NCORES)

    xt_d = nc.declare_dram_parameter("xt", [KCH, 128, N], f8, isOutput=False)
    xts_d = nc.declare_dram_parameter(
        "xts", [KCH, 128, ROWS_PER_CORE], f8, isOutput=False
    )
    sqj_d = nc.declare_dram_parameter("sqj", [128, NJT], f32, isOutput=False)
    bigdiag_d = nc.declare_dram_parameter("bigdiag", [4, 128, 512], f16, isOutput=False)
    if DEV_FOLD:
        minred_d = nc.declare_dram_parameter("minred", [128, 8], f32, isOutput=True)
    else:
        rowacc_d = nc.declare_dram_parameter(
            "rowacc", [128, IT, 512], f32, isOutput=True
        )

    with tile.TileContext(nc) as tc:
        with (
            tc.tile_pool(name="const", bufs=1) as cpool,
            tc.tile_pool(name="psum", bufs=3, space="PSUM") as psum_pool,
            tc.tile_pool(name="tpsum", bufs=2, space="PSUM") as tpsum_pool,
        ):
            # --- persistent SBUF tiles ---
            xts_t = cpool.tile([128, KCH, ROWS_PER_CORE], f8, tag="xts")
            sqj_t = cpool.tile([128, NJT], f32, tag="sqj")
            xt_t = [
                cpool.tile([128, KCH, 2048], f8, tag=f"xt{q}", name=f"xt{q}")
                for q in range(QB)
            ]
            bigdiag_t = [
                cpool.tile([128, 512], f16, tag=f"bd{b}", name=f"bd{b}")
                for b in range(4)
            ]

            def load_inputs():
                # consumption order: q=0 column block goes ahead of everything
                # bulky so the first matmul group starts ~2us in; bigdiag is
                # needed by the first STT group (JT=0), so it follows xt0.
                for k in range(KCH):
                    nc.sync.dma_start(xts_t[:, k], xts_d[k])
                nc.sync.dma_start(sqj_t[:], sqj_d[:])
                for k in range(KCH):
                    nc.sync.dma_start(xt_t[0][:, k], xt_d[k, :, 0:2048])
                for b in range(4):
                    nc.sync.dma_start(bigdiag_t[b][:], bigdiag_d[b])
                for q in range(1, QB):
                    for k in range(KCH):
                        nc.sync.dma_start(
                            xt_t[q][:, k], xt_d[k, :, q * 2048 : (q + 1) * 2048]
                        )

            if not DMA_IN_LOOP:
                load_inputs()

            rowacc = cpool.tile([128, IT, 512], f16, tag="rowacc")
            if DEV_FOLD:
                ident = cpool.tile([128, 128], f16, tag="ident")
                make_identity(nc, ident[:])
                minred_t = cpool.tile([128, 8], f32, tag="minredt")

            # --- main compute ---
            def body(_i=None):
                if DMA_IN_LOOP:
                    load_inputs()
                nc.vector.memset(rowacc[:], ACC_INIT)
                for q in range(QB):
                    for jt in range(JT_PER_Q):
                        JT = q * JT_PER_Q + jt
                        # one two-bank psum tile per j-tile group: both i-tiles
                        # side by side, consumed by a single wide STT below.
                        ps = psum_pool.tile([128, IT, 512], f32, tag="ps")
                        if USE_DR:
                            for kp in range(KP):
                                lhs = xt_t[q][
                                    :, 2 * kp : 2 * kp + 2, jt * 128 : (jt + 1) * 128
                                ]
                                for it in range(IT):
                                    nc.tensor.matmul(
                                        ps[:, it],
                                        lhs,
                                        xts_t[
                                            :,
                                            2 * kp : 2 * kp + 2,
                                            it * 512 : (it + 1) * 512,
                                        ],
                                        start=(kp == 0),
                                        stop=(kp == KP - 1),
                                        perf_mode=mybir.MatmulPerfMode.DoubleRow,
                                    )
                        else:
                            for k in range(KCH):
                                lhs = xt_t[q][:, k, jt * 128 : (jt + 1) * 128]
                                for it in range(IT):
                                    nc.tensor.matmul(
                                        ps[:, it],
                                        lhs,
                                        xts_t[:, k, it * 512 : (it + 1) * 512],
                                        start=(k == 0),
                                        stop=(k == KCH - 1),
                                    )
                        if PROBE_PE:
                            if JT % 8 == 7:
                                # minimal psum consumer so tiles recycle
                                nc.vector.scalar_tensor_tensor(
                                    rowacc[:],
                                    ps[:],
                                    sqj_t[:, JT : JT + 1],
                                    rowacc[:],
                                    op0=mybir.AluOpType.add,
                                    op1=mybir.AluOpType.min,
                                )
                            continue
                        # self-distance: +BIG on the diagonal psum slice
                        if JT < 8:
                            nc.vector.tensor_tensor(
                                ps[:, JT // 4],
                                ps[:, JT // 4],
                                bigdiag_t[JT % 4][:],
                                op=mybir.AluOpType.add,
                            )
                        nc.vector.scalar_tensor_tensor(
                            rowacc[:],
                            ps[:],
                            sqj_t[:, JT : JT + 1],
                            rowacc[:],
                            op0=mybir.AluOpType.add,
                            op1=mybir.AluOpType.min,
                        )

                if DEV_FOLD:
                    for chunk in range(8):
                        it, cc = divmod(chunk, 4)
                        tp = tpsum_pool.tile([128, 128], f16, tag="tp", name="tp")
                        nc.tensor.transpose(
                            tp[:], rowacc[:, it, cc * 128 : (cc + 1) * 128], ident[:]
                        )
                        nc.vector.tensor_reduce(
                            minred_t[:, chunk : chunk + 1],
                            tp[:],
                            axis=mybir.AxisListType.X,
                            op=mybir.AluOpType.min,
                        )

            if reps == 1:
                body()
            else:
                with tc.For_i(0, reps, 1) as _i:
                    body(_i)

            if DEV_FOLD:
                nc.sync.dma_start(minred_d[:], minred_t[:])
            else:
                acc_f32 = cpool.tile([128, IT, 512], f32, tag="accf32")
                nc.vector.tensor_copy(acc_f32[:], rowacc[:])
                nc.sync.dma_start(rowacc_d[:], acc_f32[:])

    nc.compile()
    return nc


def make_in_maps(x: np.ndarray):
    import ml_dtypes

    f8 = ml_dtypes.float8_e4m3
    sq = np.einsum("nd,nd->n", x.astype(np.float64), x.astype(np.float64)).astype(
        np.float32
    )  # [N]
    xt8 = np.ascontiguousarray(x.T).astype(f8)  # [D, N]

    bigdiag = np.zeros((4, 128, 512), ml_dtypes.float16 if hasattr(ml_dtypes, "float16") else np.float16)
    for b in range(4):
        bigdiag[b, np.arange(128), b * 128 + np.arange(128)] = BIG

    in_maps = []
    for c in range(NCORES):
        shift = c * ROWS_PER_CORE
        xt_rot = np.ascontiguousarray(np.roll(xt8, -shift, axis=1)).reshape(
            KCH, 128, N
        )
        sq_rot = np.roll(sq, -shift)
        # sqj[p, JT] = sq of global row for local column JT*128+p
        sqj = np.ascontiguousarray(sq_rot.reshape(NJT, 128).T)
        xts = (
            np.ascontiguousarray((-2.0 * x[shift : shift + ROWS_PER_CORE].T))
            .astype(f8)
            .reshape(KCH, 128, ROWS_PER_CORE)
        )
        in_maps.append({"xt": xt_rot, "xts": xts, "sqj": sqj, "bigdiag": bigdiag})
    return in_maps, sq


# back-compat aliases for the timing harness
_build_nc = build_nc
_make_in_maps = make_in_maps


def kernel(student_output: np.ndarray) -> np.ndarray:
    global _NC, LAST

    x = np.asarray(student_output, dtype=np.float32)
    assert x.shape == (N, D)
    in_maps, sq = make_in_maps(x)

    if _NC is None:
        _NC = build_nc()

    res = run_bass_kernel_spmd(_NC, in_maps, list(range(NCORES)), trace=TRACE)
    LAST = res
    results = res.results

    percore = []
    for c in range(NCORES):
        if DEV_FOLD:
            mr = np.asarray(results[c]["minred"], np.float32)  # [128, 8]
            percore.append(mr.T.reshape(-1))  # local i = chunk*128 + p
        else:
            ra = np.asarray(results[c]["rowacc"], np.float32)  # [128, IT, 512]
            # min over j-partitions: candidate[p, it, f] -> local i = it*512+f
            percore.append(ra.min(axis=0).reshape(-1))
    mins = np.concatenate(percore)  # [N] ordered by global row
    d2 = np.maximum(mins.astype(np.float64) + sq.astype(np.float64), 0.0)
    val = -np.mean(np.log(np.sqrt(d2) + EPS))
    return np.array(val, dtype=np.float32)
